# revision 51
# baseline (speedup 1.0000x reference)
"""Trainium2 Bass kernel for a dense transformer block (pre-LN, 12 heads, MLP 4x).

Strategy: data-parallel over batch across the 8 NeuronCores (B=8 -> one batch
element per core, no collectives). Per core, v2 (fp8 DoubleRow):

  - residual stream token-major fp32 [128 tok x 768] (8 token chunks)
  - LN on DVE via bn_stats/bn_aggr; LN affine params folded into the weights
  - h (LN1 out) stored feature-major in e4m3; QKV/V/proj/fc2 matmuls run in
    fp8 DoubleRow mode (contraction pairs packed in the free dim -> 2x K per
    pass); S and fc1 stay bf16 for accuracy (rel-err budget).
  - attention computed transposed: S_t[k,q] = k_fm.T @ q_fm, head pairs in
    disjoint PE row groups; exp on ACT with output scaled by 2^EXP_K (folded
    into the exp bias) and stored e4m3 so the ctx matmul can run DoubleRow;
    the 2^EXP_K cancels in the softmax normalization.
  - fc2 weights scaled by 64 on host (fp8 subnormal avoidance); descale is
    fused into the PSUM evacuation.
"""

from contextlib import ExitStack

import numpy as np

import concourse.bass as bass
import concourse.mybir as mybir
import concourse.tile as tile
from concourse import bacc
from concourse.masks import make_identity

DIM = 768
HEADS = 12
HD = 64  # head dim
HIDDEN = 3072
N_TOK = 1024
TC = N_TOK // 128  # 8 token chunks
FC = DIM // 128  # 6 feature chunks
MC_H = HIDDEN // 128  # 24 hidden chunks
EPS = 1e-5
SCALE = HD ** -0.5
EXP_K = 4  # exps scaled by 2^EXP_K (cancels in softmax norm)
FC2_WS = 64.0  # fc2 weight scale (descale fused in evacuation)
FC1_WS = 64.0  # fc1 weight scale (descale fused into the gelu)
FILLER = 5  # keep-warm matmuls per S/exp group
VPAD = 80  # padded per-(head,chunk) v stride, 16B-aligned for DoubleRow APs

F32 = mybir.dt.float32
BF16 = mybir.dt.bfloat16
F8 = mybir.dt.float8e4
DR = mybir.MatmulPerfMode.DoubleRow


def _ln_chunk(nc, stat_pool, eps_tile, zero_tile, x_ap, out_ap):
    """out = (x - mean(x)) * rsqrt(var(x) + eps), row-wise over 768."""
    stats = stat_pool.tile([128, 2, 6], F32, tag="ln_stats")
    nc.vector.bn_stats(out=stats[:, 0, :], in_=x_ap[:, 0:512])
    nc.vector.bn_stats(out=stats[:, 1, :], in_=x_ap[:, 512:768])
    mv = stat_pool.tile([128, 2], F32, tag="ln_mv")
    nc.vector.bn_aggr(out=mv, in_=stats)
    rstd = stat_pool.tile([128, 1], F32, tag="ln_rstd")
    nc.scalar.activation(
        out=rstd, in_=mv[:, 1:2], func=mybir.ActivationFunctionType.Sqrt,
        bias=eps_tile, scale=1.0,
    )
    nc.vector.reciprocal(out=rstd, in_=rstd)
    nc.vector.tensor_scalar(
        out=out_ap, in0=x_ap, scalar1=mv[:, 0:1], scalar2=rstd,
        op0=mybir.AluOpType.subtract, op1=mybir.AluOpType.mult,
    )


def _ln_chunk_act(nc, stat_pool, eps_tile, x_ap, out_ap):
    """LN with row sums/sumsq from ACT accumulators; DVE does only the tiny
    stat math + the normalize. Used where the DVE is the regional bottleneck
    and the ACT is idle. No cancellation risk: |mean| << std for LN inputs.
    The accum passes dump their elementwise output into out_ap, which the
    final normalize overwrites."""
    scr = out_ap
    sx = stat_pool.tile([128, 1], F32, tag="ln_sx")
    sxx = stat_pool.tile([128, 1], F32, tag="ln_sxx")
    nc.scalar.activation(out=scr, in_=x_ap,
                         func=mybir.ActivationFunctionType.Copy, accum_out=sx)
    nc.scalar.activation(out=scr, in_=x_ap,
                         func=mybir.ActivationFunctionType.Square,
                         accum_out=sxx)
    mean = stat_pool.tile([128, 1], F32, tag="ln_mean")
    nc.vector.tensor_scalar_mul(out=mean, in0=sx, scalar1=1.0 / DIM)
    m2 = stat_pool.tile([128, 1], F32, tag="ln_m2")
    nc.vector.tensor_tensor(out=m2, in0=mean, in1=mean,
                            op=mybir.AluOpType.mult)
    var = stat_pool.tile([128, 1], F32, tag="ln_var")
    nc.vector.tensor_scalar(
        out=var, in0=sxx, scalar1=1.0 / DIM, scalar2=m2,
        op0=mybir.AluOpType.mult, op1=mybir.AluOpType.subtract)
    rstd = stat_pool.tile([128, 1], F32, tag="ln_rstd")
    nc.scalar.activation(
        out=rstd, in_=var, func=mybir.ActivationFunctionType.Sqrt,
        bias=eps_tile, scale=1.0,
    )
    nc.vector.reciprocal(out=rstd, in_=rstd)
    nc.vector.tensor_scalar(
        out=out_ap, in0=x_ap, scalar1=mean, scalar2=rstd,
        op0=mybir.AluOpType.subtract, op1=mybir.AluOpType.mult,
    )


class TileCtx:
    """TileContext + an ExitStack, flattened to dodge the nested-block limit."""

    def __init__(self, nc):
        self.st = ExitStack()
        self.nc = nc

    def __enter__(self):
        tc = self.st.enter_context(tile.TileContext(self.nc))
        return tc, self.st

    def __exit__(self, *exc):
        return self.st.__exit__(*exc)


class _Pools:
    NAMES = ()

    def __init__(self, tc):
        self.st = ExitStack()
        self.tc = tc

    def __enter__(self):
        return tuple(self.st.enter_context(self.tc.tile_pool(name=n, bufs=b))
                     for n, b in self.NAMES)

    def __exit__(self, *exc):
        return self.st.__exit__(*exc)


class AttnPools(_Pools):
    NAMES = (("qk", 1), ("vaug", 1), ("ctxfm", 1), ("wproj", 1))


class QkvPools(_Pools):
    NAMES = (("hfm", 1), ("wqkv", 3), ("wvp", 1), ("exps", 15))


def build_bass():
    nc = bacc.Bacc("TRN2", debug=False)

    x_d = nc.dram_tensor("x", [N_TOK, DIM], F32, kind="ExternalInput")
    qkv_wt_d = nc.dram_tensor("qkv_wt", [DIM, 3 * DIM], F8, kind="ExternalInput")
    qkb_pm_d = nc.dram_tensor("qkb_pm", [128, 2 * FC], F32, kind="ExternalInput")
    vb_d = nc.dram_tensor("vb", [DIM], F32, kind="ExternalInput")
    proj_wt_d = nc.dram_tensor("proj_wt", [DIM, DIM], F8, kind="ExternalInput")
    projb_d = nc.dram_tensor("projb", [DIM], BF16, kind="ExternalInput")
    fc1_wt_d = nc.dram_tensor("fc1_wt", [DIM, HIDDEN], F8, kind="ExternalInput")
    fc1b_pm_d = nc.dram_tensor("fc1b_pm", [128, MC_H], F32, kind="ExternalInput")
    fc2_wt_d = nc.dram_tensor("fc2_wt", [HIDDEN, DIM], F8, kind="ExternalInput")
    fc2b64_d = nc.dram_tensor("fc2b64", [DIM], BF16, kind="ExternalInput")
    out_d = nc.dram_tensor("out", [N_TOK, DIM], F32, kind="ExternalOutput")

    x_dt = x_d.ap().rearrange("(t p) c -> p t c", p=128)
    out_dt = out_d.ap().rearrange("(t p) c -> p t c", p=128)
    # weight chunk views: [128 part of in-feat, in-chunk, out-col]
    qkv_w3 = qkv_wt_d.ap().rearrange("(ko p) n -> p ko n", p=128)
    proj_w3 = proj_wt_d.ap().rearrange("(ko p) n -> p ko n", p=128)
    fc1_w3 = fc1_wt_d.ap().rearrange("(ko p) n -> p ko n", p=128)
    fc2_w3 = fc2_wt_d.ap().rearrange("(ko p) n -> p ko n", p=128)

    def bcast128(ap_1d, n):
        return bass.AP(tensor=ap_1d.tensor, offset=ap_1d.offset,
                       ap=[[0, 128], [1, n]])

    with TileCtx(nc) as (tc, st):
        if True:
            const_pool = st.enter_context(tc.tile_pool(name="const", bufs=1))
            resid_pool = st.enter_context(tc.tile_pool(name="resid", bufs=1))
            stat_pool = st.enter_context(tc.tile_pool(name="stats", bufs=2))
            dsm_pool = st.enter_context(tc.tile_pool(name="dsm", bufs=2))
            bexp_pool = st.enter_context(tc.tile_pool(name="bexp", bufs=1))
            # PSUM: big (S tiles [128,1024] f32 = 2 banks; fc1; transposes),
            # small 1-bank (qkv/v/proj/fc2), ctx [65,512]. 2*2+2+2 = 8 banks.
            psum_big = st.enter_context(
                tc.tile_pool(name="psum_big", bufs=2, space="PSUM"))
            psum_small = st.enter_context(
                tc.tile_pool(name="psum_small", bufs=2, space="PSUM"))
            psum_ctx = st.enter_context(
                tc.tile_pool(name="psum_ctx", bufs=2, space="PSUM"))
            h2fm_pool = st.enter_context(tc.tile_pool(name="h2fm", bufs=1))
            g_pool = st.enter_context(tc.tile_pool(name="gfm", bufs=1))
            wfc2_pool = st.enter_context(tc.tile_pool(name="wfc2", bufs=1))
            out_pool = st.enter_context(tc.tile_pool(name="outt", bufs=2))

            x_sb = resid_pool.tile([128, TC, DIM], F32)
            wv_sb = const_pool.tile([128, 2, FC, 384], F8)
            for t in range(2):
                nc.sync.dma_start(out=x_sb[:, t, :], in_=x_dt[:, t, :])
            for nv in range(2):
                nc.sync.dma_start(
                    out=wv_sb[:, nv],
                    in_=qkv_w3[:, :, 2 * DIM + nv * 384:2 * DIM + (nv + 1) * 384])
            for t in range(2, TC):
                nc.sync.dma_start(out=x_sb[:, t, :], in_=x_dt[:, t, :])
            ident = const_pool.tile([128, 128], BF16)
            make_identity(nc, ident)
            eps_tile = const_pool.tile([128, 1], F32)
            nc.vector.memset(eps_tile, EPS)
            vb_bc = const_pool.tile([128, DIM], F32)
            nc.sync.dma_start(out=vb_bc, in_=bcast128(vb_d.ap(), DIM))
            qkb_pm = const_pool.tile([128, 2 * FC], F32)
            nc.sync.dma_start(out=qkb_pm, in_=qkb_pm_d.ap())
            fc1b_pm = const_pool.tile([128, MC_H], F32)
            nc.sync.dma_start(out=fc1b_pm, in_=fc1b_pm_d.ap())
            ones_bf = const_pool.tile([128, 128], BF16)
            nc.vector.memset(ones_bf, 1.0)
            zero_tile = const_pool.tile([128, 1], F32)
            nc.vector.memset(zero_tile, 0.0)
            expk_tile = const_pool.tile([128, 1], F32)
            nc.vector.memset(expk_tile, float(EXP_K * np.log(2.0)))
            projb_row = const_pool.tile([1, DIM], BF16)
            fc2b_row = const_pool.tile([1, DIM], BF16)

            def row_ap(ap_1d, n):
                return bass.AP(tensor=ap_1d.tensor, offset=ap_1d.offset,
                               ap=[[0, 1], [1, n]])

            h2_fm = h2fm_pool.tile([128, FC, N_TOK], F8, tag="hfm2")
            g_fm = g_pool.tile([128, MC_H, N_TOK], F8)
            wf2 = wfc2_pool.tile([128, MC_H, DIM], F8)

            def ln_transpose(t, dst_fm, evac_act=False, ln_act=False):
                """LN of token chunk t + PE-transpose into dst_fm[:, :, t*128:]."""
                h_tile = stat_pool.tile([128, DIM], BF16, tag="h_tile")
                _ln_chunk(nc, stat_pool, eps_tile, zero_tile,
                          x_sb[:, t, :], h_tile)
                tr = psum_big.tile([128, FC, 128], BF16, tag="big")
                for f in range(FC):
                    nc.tensor.transpose(
                        tr[:, f, :], h_tile[:, f * 128:(f + 1) * 128], ident)
                if evac_act:
                    nc.scalar.copy(
                        out=dst_fm[:, :, t * 128:(t + 1) * 128], in_=tr)
                else:
                    nc.vector.tensor_copy(
                        out=dst_fm[:, :, t * 128:(t + 1) * 128], in_=tr)

            wfc1_pool = st.enter_context(tc.tile_pool(name="wfc1", bufs=2))

            # ============ attention region (qkv + attention + proj) =========
            with AttnPools(tc) as (qk_pool, v_pool, ctx_pool, wproj_pool):
                qk_fm = qk_pool.tile([128, 2 * FC, N_TOK], BF16)
                # v: [128 key, head, key-chunk, 64+1(den)] padded to VPAD for
                # 16B-aligned DoubleRow pair strides
                v_aug = v_pool.tile([128, HEADS, TC, VPAD], F8)
                ctx_fm = ctx_pool.tile([128, FC, N_TOK], F8)
                wp = wproj_pool.tile([128, FC, DIM], F8)

                with QkvPools(tc) as (hfm_pool, wqkv_pool, wv_pool, exps_pool):
                    h_fm = hfm_pool.tile([128, FC, N_TOK], F8, tag="hfm")

                    wv = [wv_sb[:, 0], wv_sb[:, 1]]

                    wq_cache = {}

                    def emit_qk_q(m, q, fast):
                        """q/k chunk m, one token half; DR when `fast`, else
                        normal-rate fp8 (identical math, 2x PE time spent
                        inside the ACT-bound exp phase to keep the HAM
                        clock-gate open)."""
                        if m not in wq_cache:
                            wt = wqkv_pool.tile([128, FC, 128], F8, tag="wqkv")
                            nc.sync.dma_start(
                                out=wt,
                                in_=qkv_w3[:, :, m * 128:(m + 1) * 128])
                            wq_cache[m] = wt
                        wt = wq_cache[m]
                        ps = psum_small.tile([128, 512], F32, tag="sm")
                        if fast:
                            for c in range(FC // 2):
                                nc.tensor.matmul(
                                    ps, wt[:, 2 * c:2 * c + 2, :],
                                    h_fm[:, 2 * c:2 * c + 2,
                                         q * 512:(q + 1) * 512],
                                    start=(c == 0), stop=(c == FC // 2 - 1),
                                    perf_mode=DR)
                        else:
                            for k in range(FC):
                                nc.tensor.matmul(
                                    ps, wt[:, k, :],
                                    h_fm[:, k, q * 512:(q + 1) * 512],
                                    start=(k == 0), stop=(k == FC - 1))
                        nc.vector.tensor_scalar_add(
                            out=qk_fm[:, m, q * 512:(q + 1) * 512], in0=ps,
                            scalar1=qkb_pm[:, m:m + 1])
                        if q == 1:
                            wq_cache.pop(m)

                    exps_ctr = [0]

                    def alloc_exps():
                        out = []
                        for ab in range(2):
                            row = []
                            for kcp in range(TC // 2):
                                exps_ctr[0] += 1
                                row.append(exps_pool.tile(
                                    [128, 2, N_TOK], F8, tag="exps",
                                    name=f"exps{exps_ctr[0]}"))
                            out.append(row)
                        return out

                    I32 = mybir.dt.int32
                    LOG2E = 1.4426950408889634

                    def emit_exp_dve(e_out, sp):
                        """exp via float-bits construction on the DVE:
                        i = int((S*SCALE*log2e + 127 + EXP_K) * 2^23);
                        bitcast(i) ~= 2^EXP_K * exp(S*SCALE) with ~2% weight
                        error after softmax normalization (under the e4m3
                        storage error already accepted). Offloads ACT, whose
                        1 elem/lane/cycle exp stream bounds the attention
                        phase."""
                        scr = bexp_pool.tile([128, N_TOK], I32, tag="bexp")
                        nc.vector.tensor_scalar(
                            out=scr.bitcast(F32), in0=sp,
                            scalar1=float(SCALE * LOG2E * (1 << 23)),
                            scalar2=float((127.0 + EXP_K) * (1 << 23)),
                            op0=mybir.AluOpType.mult,
                            op1=mybir.AluOpType.add)
                        nc.vector.tensor_copy(out=scr, in_=scr.bitcast(F32))
                        nc.vector.tensor_copy(out=e_out, in_=scr.bitcast(F32))

                    def emit_s_exp_part(j, exps, kcp_list, q_list):
                        """S + exp for head pair (2j, 2j+1), restricted to kc
                        pairs `kcp_list` and token halves `q_list`. Half-wide
                        calls let pair 0 start as soon as LN chunks 0-3 are
                        done instead of waiting for the full LN1 sweep."""
                        for kcp in kcp_list:
                            for ab in range(2):
                                e_t = exps[ab][kcp]
                                po = 64 * ab
                                for sub in range(2):
                                    kc = 2 * kcp + sub
                                    if len(q_list) == 2:
                                        sp = psum_big.tile([128, N_TOK], F32,
                                                           tag="big")
                                        for q in q_list:
                                            nc.tensor.matmul(
                                                sp[:, q * 512:(q + 1) * 512],
                                                qk_fm[po:po + 64, 6 + j,
                                                      kc * 128:(kc + 1) * 128],
                                                qk_fm[po:po + 64, j,
                                                      q * 512:(q + 1) * 512],
                                                start=True, stop=True)
                                        if ((1 <= j <= 4 and kcp == 3
                                                and (ab == 1 or sub == 1))
                                                or (j == 5 and kcp == 0
                                                    and (ab == 1 or sub == 1))):
                                            emit_exp_dve(e_t[:, sub, :], sp)
                                        else:
                                            nc.scalar.activation(
                                                out=e_t[:, sub, :], in_=sp,
                                                func=mybir.ActivationFunctionType.Exp,
                                                scale=SCALE, bias=expk_tile)
                                    else:
                                        q = q_list[0]
                                        sp = psum_big.tile([128, 512], F32,
                                                           tag="big")
                                        nc.tensor.matmul(
                                            sp,
                                            qk_fm[po:po + 64, 6 + j,
                                                  kc * 128:(kc + 1) * 128],
                                            qk_fm[po:po + 64, j,
                                                  q * 512:(q + 1) * 512],
                                            start=True, stop=True)
                                        nc.scalar.activation(
                                            out=e_t[:, sub,
                                                    q * 512:(q + 1) * 512],
                                            in_=sp,
                                            func=mybir.ActivationFunctionType.Exp,
                                            scale=SCALE, bias=expk_tile)

                    # LN1 stats front pass: all bn_stats/aggr on DVE, then
                    # ONE batched Sqrt + ONE batched reciprocal, so every
                    # Sqrt precedes the first attention Exp (one ACT
                    # table-set switch instead of per-chunk thrash)
                    lnstat = const_pool.tile([128, TC, 3], F32)
                    for t in range(TC):
                        stats = stat_pool.tile([128, 2, 6], F32,
                                               tag="ln_stats")
                        nc.vector.bn_stats(out=stats[:, 0, :],
                                           in_=x_sb[:, t, 0:512])
                        nc.vector.bn_stats(out=stats[:, 1, :],
                                           in_=x_sb[:, t, 512:768])
                        nc.vector.bn_aggr(out=lnstat[:, t, 0:2], in_=stats)
                    nc.scalar.activation(
                        out=lnstat[:, :, 2:3], in_=lnstat[:, :, 1:2],
                        func=mybir.ActivationFunctionType.Sqrt,
                        bias=eps_tile, scale=1.0)
                    nc.vector.reciprocal(out=lnstat[:, :, 2:3],
                                         in_=lnstat[:, :, 2:3])

                    # LN1 normalize + transpose + v per token chunk
                    for t in range(TC):
                        h_tile = stat_pool.tile([128, DIM], BF16,
                                                tag="h_tile")
                        nc.vector.tensor_scalar(
                            out=h_tile, in0=x_sb[:, t, :],
                            scalar1=lnstat[:, t, 0:1],
                            scalar2=lnstat[:, t, 2:3],
                            op0=mybir.AluOpType.subtract,
                            op1=mybir.AluOpType.mult)
                        tr = psum_big.tile([128, FC, 128], BF16, tag="big")
                        for f in range(FC):
                            nc.tensor.transpose(
                                tr[:, f, :], h_tile[:, f * 128:(f + 1) * 128],
                                ident)
                        nc.scalar.copy(
                            out=h_fm[:, :, t * 128:(t + 1) * 128], in_=tr)
                        for nv in range(2):
                            ps = psum_small.tile([128, 384], F32, tag="sm")
                            for c in range(FC // 2):
                                nc.tensor.matmul(
                                    ps,
                                    h_fm[:, 2 * c:2 * c + 2,
                                         t * 128:(t + 1) * 128],
                                    wv[nv][:, 2 * c:2 * c + 2, :],
                                    start=(c == 0), stop=(c == FC // 2 - 1),
                                    perf_mode=DR)
                            nc.vector.tensor_tensor(
                                out=v_aug[:, nv * 6:(nv + 1) * 6, t, 0:HD],
                                in0=ps.rearrange("p (h d) -> p h d", d=HD),
                                in1=vb_bc[:, nv * 384:(nv + 1) * 384].rearrange(
                                    "p (h d) -> p h d", d=HD),
                                op=mybir.AluOpType.add)
                        if t == 3:
                            emit_qk_q(0, 0, True)
                            emit_qk_q(6, 0, True)
                            exps0 = alloc_exps()
                            emit_s_exp_part(0, exps0, (0, 1), (0,))
                    nc.vector.memset(v_aug[:, :, :, HD], 1.0)

                    def emit_ctx(j, exps):
                        for ab in range(2):
                            h = 2 * j + ab
                            po = 64 * ab
                            for q in range(2):
                                cp = psum_ctx.tile([65, 512], F32, tag="ctx")
                                for kcp in range(TC // 2):
                                    nc.tensor.matmul(
                                        cp,
                                        v_aug[:, h, 2 * kcp:2 * kcp + 2, 0:65],
                                        exps[ab][kcp][:, :,
                                                      q * 512:(q + 1) * 512],
                                        start=(kcp == 0),
                                        stop=(kcp == TC // 2 - 1),
                                        perf_mode=DR)
                                den = dsm_pool.tile([1, 512], F32, tag="den")
                                nc.vector.tensor_copy(out=den,
                                                      in_=cp[64:65, :])
                                scr = dsm_pool.tile([1, 512], F32, tag="scr")
                                rec = dsm_pool.tile([1, 512], F32, tag="rec")
                                nc.vector.reciprocal_approx_accurate(
                                    out=rec, in_=den, scratch=scr)
                                bcd = dsm_pool.tile([128, 512], F32, tag="bcd")
                                nc.gpsimd.partition_broadcast(bcd, rec)
                                if ab == 0:
                                    nc.vector.tensor_tensor(
                                        out=ctx_fm[0:64, j,
                                                   q * 512:(q + 1) * 512],
                                        in0=cp[0:64, :], in1=bcd[0:64, :],
                                        op=mybir.AluOpType.mult)
                                else:
                                    cu = dsm_pool.tile([128, 512], BF16,
                                                       tag="cu")
                                    nc.vector.tensor_copy(out=cu[po:po + 64, :],
                                                          in_=cp[0:64, :])
                                    nc.vector.tensor_tensor(
                                        out=ctx_fm[po:po + 64, j,
                                                   q * 512:(q + 1) * 512],
                                        in0=cu[po:po + 64, :],
                                        in1=bcd[po:po + 64, :],
                                        op=mybir.AluOpType.mult)

                    pend = None
                    for j in range(6):
                        if j == 0:
                            emit_qk_q(0, 1, True)
                            emit_qk_q(6, 1, True)
                        else:
                            emit_qk_q(j, 0, True)
                            emit_qk_q(j, 1, True)
                            emit_qk_q(6 + j, 0, True)
                            emit_qk_q(6 + j, 1, True)
                        if j == 3:
                            nc.sync.dma_start(out=wp, in_=proj_w3)
                            nc.sync.dma_start(out=projb_row,
                                              in_=row_ap(projb_d.ap(), DIM))
                        if j == 4:
                            nc.sync.dma_start(out=wf2, in_=fc2_w3)
                            nc.sync.dma_start(out=fc2b_row,
                                              in_=row_ap(fc2b64_d.ap(), DIM))
                        if j == 0:
                            emit_s_exp_part(0, exps0, (0, 1), (1,))
                            emit_s_exp_part(0, exps0, (2, 3), (0, 1))
                            exps = exps0
                        else:
                            exps = alloc_exps()
                            emit_s_exp_part(j, exps, (0, 1, 2, 3), (0, 1))
                        if pend is not None:
                            emit_ctx(*pend)
                        pend = (j, exps)
                    emit_ctx(*pend)

                # keep-warm bridge: PE idles ~3us on the last ctx evacuation
                # chain; stay busy so HAM holds 2.4 GHz into the proj sweep
                wba = psum_big.tile([128, N_TOK], F32, tag="big")
                for _ in range(8):
                    nc.tensor.matmul(wba[:, 0:512], ident, qk_fm[:, 0, 0:512],
                                     start=True, stop=True)

                # ---------------- proj + residual + LN2 ----------------
                for t in range(TC):
                    for nv in range(2):
                        ps = psum_small.tile([128, 384], F32, tag="sm")
                        # normal-rate fp8 (not DR): this region is gated by
                        # the DVE LN2/evac chain; the extra PE time keeps HAM
                        # from re-throttling into the fc1 sweep
                        for k in range(FC):
                            nc.tensor.matmul(
                                ps,
                                ctx_fm[:, k, t * 128:(t + 1) * 128],
                                wp[:, k, nv * 384:(nv + 1) * 384],
                                start=(k == 0), stop=False)
                        sl = slice(nv * 384, (nv + 1) * 384)
                        nc.tensor.matmul(
                            ps, ones_bf[0:1, :], projb_row[0:1, sl],
                            start=False, stop=True)
                        nc.vector.tensor_add(
                            out=x_sb[:, t, sl], in0=ps, in1=x_sb[:, t, sl])
                    ln_transpose(t, h2_fm, evac_act=True)

                # (warm bridge) the PE would idle >3.4us here waiting for the
                # LN2 tail, dropping HAM to 1.2 GHz for the start of fc1;
                # burn the wait on throwaway matmuls instead
                wb = psum_big.tile([128, N_TOK], F32, tag="big")
                for _ in range(16):
                    nc.tensor.matmul(wb[:, 0:512], ident, qk_fm[:, 0, 0:512],
                                     start=True, stop=True)

            # ------- MLP: fc1 (fp8 DR, x64 weights; batched fetches) -------
            if True:
                for m4 in range(MC_H // 4):
                    w1t = wfc1_pool.tile([128, FC, 512], F8, tag="w1t")
                    nc.sync.dma_start(
                        out=w1t, in_=fc1_w3[:, :, m4 * 512:(m4 + 1) * 512])
                    for mi in range(4):
                        m = 4 * m4 + mi
                        ps = psum_big.tile([128, N_TOK], F32, tag="big")
                        for half in range(2):
                            for c in range(FC // 2):
                                nc.tensor.matmul(
                                    ps[:, half * 512:(half + 1) * 512],
                                    w1t[:, 2 * c:2 * c + 2,
                                        mi * 128:(mi + 1) * 128],
                                    h2_fm[:, 2 * c:2 * c + 2,
                                          half * 512:(half + 1) * 512],
                                    start=(c == 0), stop=(c == FC // 2 - 1),
                                    perf_mode=DR)
                        nc.scalar.activation(
                            out=g_fm[:, m, :], in_=ps,
                            func=mybir.ActivationFunctionType.Gelu,
                            bias=fc1b_pm[:, m:m + 1], scale=1.0 / FC1_WS)




# revision 52
# speedup vs baseline: 1.0039x; 1.0039x over previous
"""Trainium2 Bass kernel for a dense transformer block (pre-LN, 12 heads, MLP 4x).

Strategy: data-parallel over batch across the 8 NeuronCores (B=8 -> one batch
element per core, no collectives). Per core, v2 (fp8 DoubleRow):

  - residual stream token-major fp32 [128 tok x 768] (8 token chunks)
  - LN on DVE via bn_stats/bn_aggr; LN affine params folded into the weights
  - h (LN1 out) stored feature-major in e4m3; QKV/V/proj/fc2 matmuls run in
    fp8 DoubleRow mode (contraction pairs packed in the free dim -> 2x K per
    pass); S and fc1 stay bf16 for accuracy (rel-err budget).
  - attention computed transposed: S_t[k,q] = k_fm.T @ q_fm, head pairs in
    disjoint PE row groups; exp on ACT with output scaled by 2^EXP_K (folded
    into the exp bias) and stored e4m3 so the ctx matmul can run DoubleRow;
    the 2^EXP_K cancels in the softmax normalization.
  - fc2 weights scaled by 64 on host (fp8 subnormal avoidance); descale is
    fused into the PSUM evacuation.
"""

from contextlib import ExitStack

import numpy as np

import concourse.bass as bass
import concourse.mybir as mybir
import concourse.tile as tile
from concourse import bacc
from concourse.masks import make_identity

DIM = 768
HEADS = 12
HD = 64  # head dim
HIDDEN = 3072
N_TOK = 1024
TC = N_TOK // 128  # 8 token chunks
FC = DIM // 128  # 6 feature chunks
MC_H = HIDDEN // 128  # 24 hidden chunks
EPS = 1e-5
SCALE = HD ** -0.5
EXP_K = 4  # exps scaled by 2^EXP_K (cancels in softmax norm)
FC2_WS = 64.0  # fc2 weight scale (descale fused in evacuation)
FC1_WS = 64.0  # fc1 weight scale (descale fused into the gelu)
FILLER = 5  # keep-warm matmuls per S/exp group
VPAD = 80  # padded per-(head,chunk) v stride, 16B-aligned for DoubleRow APs

F32 = mybir.dt.float32
BF16 = mybir.dt.bfloat16
F8 = mybir.dt.float8e4
DR = mybir.MatmulPerfMode.DoubleRow


def _ln_chunk(nc, stat_pool, eps_tile, zero_tile, x_ap, out_ap):
    """out = (x - mean(x)) * rsqrt(var(x) + eps), row-wise over 768."""
    stats = stat_pool.tile([128, 2, 6], F32, tag="ln_stats")
    nc.vector.bn_stats(out=stats[:, 0, :], in_=x_ap[:, 0:512])
    nc.vector.bn_stats(out=stats[:, 1, :], in_=x_ap[:, 512:768])
    mv = stat_pool.tile([128, 2], F32, tag="ln_mv")
    nc.vector.bn_aggr(out=mv, in_=stats)
    rstd = stat_pool.tile([128, 1], F32, tag="ln_rstd")
    nc.scalar.activation(
        out=rstd, in_=mv[:, 1:2], func=mybir.ActivationFunctionType.Sqrt,
        bias=eps_tile, scale=1.0,
    )
    nc.vector.reciprocal(out=rstd, in_=rstd)
    nc.vector.tensor_scalar(
        out=out_ap, in0=x_ap, scalar1=mv[:, 0:1], scalar2=rstd,
        op0=mybir.AluOpType.subtract, op1=mybir.AluOpType.mult,
    )


def _ln_chunk_act(nc, stat_pool, eps_tile, x_ap, out_ap):
    """LN with row sums/sumsq from ACT accumulators; DVE does only the tiny
    stat math + the normalize. Used where the DVE is the regional bottleneck
    and the ACT is idle. No cancellation risk: |mean| << std for LN inputs.
    The accum passes dump their elementwise output into out_ap, which the
    final normalize overwrites."""
    scr = out_ap
    sx = stat_pool.tile([128, 1], F32, tag="ln_sx")
    sxx = stat_pool.tile([128, 1], F32, tag="ln_sxx")
    nc.scalar.activation(out=scr, in_=x_ap,
                         func=mybir.ActivationFunctionType.Copy, accum_out=sx)
    nc.scalar.activation(out=scr, in_=x_ap,
                         func=mybir.ActivationFunctionType.Square,
                         accum_out=sxx)
    mean = stat_pool.tile([128, 1], F32, tag="ln_mean")
    nc.vector.tensor_scalar_mul(out=mean, in0=sx, scalar1=1.0 / DIM)
    m2 = stat_pool.tile([128, 1], F32, tag="ln_m2")
    nc.vector.tensor_tensor(out=m2, in0=mean, in1=mean,
                            op=mybir.AluOpType.mult)
    var = stat_pool.tile([128, 1], F32, tag="ln_var")
    nc.vector.tensor_scalar(
        out=var, in0=sxx, scalar1=1.0 / DIM, scalar2=m2,
        op0=mybir.AluOpType.mult, op1=mybir.AluOpType.subtract)
    rstd = stat_pool.tile([128, 1], F32, tag="ln_rstd")
    nc.scalar.activation(
        out=rstd, in_=var, func=mybir.ActivationFunctionType.Sqrt,
        bias=eps_tile, scale=1.0,
    )
    nc.vector.reciprocal(out=rstd, in_=rstd)
    nc.vector.tensor_scalar(
        out=out_ap, in0=x_ap, scalar1=mean, scalar2=rstd,
        op0=mybir.AluOpType.subtract, op1=mybir.AluOpType.mult,
    )


class TileCtx:
    """TileContext + an ExitStack, flattened to dodge the nested-block limit."""

    def __init__(self, nc):
        self.st = ExitStack()
        self.nc = nc

    def __enter__(self):
        tc = self.st.enter_context(tile.TileContext(self.nc))
        return tc, self.st

    def __exit__(self, *exc):
        return self.st.__exit__(*exc)


class _Pools:
    NAMES = ()

    def __init__(self, tc):
        self.st = ExitStack()
        self.tc = tc

    def __enter__(self):
        return tuple(self.st.enter_context(self.tc.tile_pool(name=n, bufs=b))
                     for n, b in self.NAMES)

    def __exit__(self, *exc):
        return self.st.__exit__(*exc)


class AttnPools(_Pools):
    NAMES = (("qk", 1), ("vaug", 1), ("ctxfm", 1), ("wproj", 1))


class QkvPools(_Pools):
    NAMES = (("hfm", 1), ("wqkv", 3), ("wvp", 1), ("exps", 15))


def build_bass():
    nc = bacc.Bacc("TRN2", debug=False)

    x_d = nc.dram_tensor("x", [N_TOK, DIM], F32, kind="ExternalInput")
    qkv_wt_d = nc.dram_tensor("qkv_wt", [DIM, 3 * DIM], F8, kind="ExternalInput")
    qkb_pm_d = nc.dram_tensor("qkb_pm", [128, 2 * FC], F32, kind="ExternalInput")
    vb_d = nc.dram_tensor("vb", [DIM], F32, kind="ExternalInput")
    proj_wt_d = nc.dram_tensor("proj_wt", [DIM, DIM], F8, kind="ExternalInput")
    projb_d = nc.dram_tensor("projb", [DIM], BF16, kind="ExternalInput")
    fc1_wt_d = nc.dram_tensor("fc1_wt", [DIM, HIDDEN], F8, kind="ExternalInput")
    fc1b_pm_d = nc.dram_tensor("fc1b_pm", [128, MC_H], F32, kind="ExternalInput")
    fc2_wt_d = nc.dram_tensor("fc2_wt", [HIDDEN, DIM], F8, kind="ExternalInput")
    fc2b64_d = nc.dram_tensor("fc2b64", [DIM], BF16, kind="ExternalInput")
    out_d = nc.dram_tensor("out", [N_TOK, DIM], F32, kind="ExternalOutput")

    x_dt = x_d.ap().rearrange("(t p) c -> p t c", p=128)
    out_dt = out_d.ap().rearrange("(t p) c -> p t c", p=128)
    # weight chunk views: [128 part of in-feat, in-chunk, out-col]
    qkv_w3 = qkv_wt_d.ap().rearrange("(ko p) n -> p ko n", p=128)
    proj_w3 = proj_wt_d.ap().rearrange("(ko p) n -> p ko n", p=128)
    fc1_w3 = fc1_wt_d.ap().rearrange("(ko p) n -> p ko n", p=128)
    fc2_w3 = fc2_wt_d.ap().rearrange("(ko p) n -> p ko n", p=128)

    def bcast128(ap_1d, n):
        return bass.AP(tensor=ap_1d.tensor, offset=ap_1d.offset,
                       ap=[[0, 128], [1, n]])

    with TileCtx(nc) as (tc, st):
        if True:
            const_pool = st.enter_context(tc.tile_pool(name="const", bufs=1))
            resid_pool = st.enter_context(tc.tile_pool(name="resid", bufs=1))
            stat_pool = st.enter_context(tc.tile_pool(name="stats", bufs=2))
            dsm_pool = st.enter_context(tc.tile_pool(name="dsm", bufs=2))
            bexp_pool = st.enter_context(tc.tile_pool(name="bexp", bufs=1))
            # PSUM: big (S tiles [128,1024] f32 = 2 banks; fc1; transposes),
            # small 1-bank (qkv/v/proj/fc2), ctx [65,512]. 2*2+2+2 = 8 banks.
            psum_big = st.enter_context(
                tc.tile_pool(name="psum_big", bufs=2, space="PSUM"))
            psum_small = st.enter_context(
                tc.tile_pool(name="psum_small", bufs=2, space="PSUM"))
            psum_ctx = st.enter_context(
                tc.tile_pool(name="psum_ctx", bufs=2, space="PSUM"))
            h2fm_pool = st.enter_context(tc.tile_pool(name="h2fm", bufs=1))
            g_pool = st.enter_context(tc.tile_pool(name="gfm", bufs=1))
            wfc2_pool = st.enter_context(tc.tile_pool(name="wfc2", bufs=1))
            out_pool = st.enter_context(tc.tile_pool(name="outt", bufs=2))

            x_sb = resid_pool.tile([128, TC, DIM], F32)
            wv_sb = const_pool.tile([128, 2, FC, 384], F8)
            for t in range(2):
                nc.sync.dma_start(out=x_sb[:, t, :], in_=x_dt[:, t, :])
            for nv in range(2):
                nc.sync.dma_start(
                    out=wv_sb[:, nv],
                    in_=qkv_w3[:, :, 2 * DIM + nv * 384:2 * DIM + (nv + 1) * 384])
            for t in range(2, TC):
                nc.sync.dma_start(out=x_sb[:, t, :], in_=x_dt[:, t, :])
            ident = const_pool.tile([128, 128], BF16)
            make_identity(nc, ident)
            eps_tile = const_pool.tile([128, 1], F32)
            nc.vector.memset(eps_tile, EPS)
            vb_bc = const_pool.tile([128, DIM], F32)
            nc.sync.dma_start(out=vb_bc, in_=bcast128(vb_d.ap(), DIM))
            qkb_pm = const_pool.tile([128, 2 * FC], F32)
            nc.sync.dma_start(out=qkb_pm, in_=qkb_pm_d.ap())
            fc1b_pm = const_pool.tile([128, MC_H], F32)
            nc.sync.dma_start(out=fc1b_pm, in_=fc1b_pm_d.ap())
            ones_bf = const_pool.tile([128, 128], BF16)
            nc.vector.memset(ones_bf, 1.0)
            zero_tile = const_pool.tile([128, 1], F32)
            nc.vector.memset(zero_tile, 0.0)
            expk_tile = const_pool.tile([128, 1], F32)
            nc.vector.memset(expk_tile, float(EXP_K * np.log(2.0)))
            projb_row = const_pool.tile([1, DIM], BF16)
            fc2b_row = const_pool.tile([1, DIM], BF16)

            def row_ap(ap_1d, n):
                return bass.AP(tensor=ap_1d.tensor, offset=ap_1d.offset,
                               ap=[[0, 1], [1, n]])

            h2_fm = h2fm_pool.tile([128, FC, N_TOK], F8, tag="hfm2")
            g_fm = g_pool.tile([128, MC_H, N_TOK], F8)
            wf2 = wfc2_pool.tile([128, MC_H, DIM], F8)

            def ln_transpose(t, dst_fm, evac_act=False, ln_act=False):
                """LN of token chunk t + PE-transpose into dst_fm[:, :, t*128:]."""
                h_tile = stat_pool.tile([128, DIM], BF16, tag="h_tile")
                _ln_chunk(nc, stat_pool, eps_tile, zero_tile,
                          x_sb[:, t, :], h_tile)
                tr = psum_big.tile([128, FC, 128], BF16, tag="big")
                for f in range(FC):
                    nc.tensor.transpose(
                        tr[:, f, :], h_tile[:, f * 128:(f + 1) * 128], ident)
                if evac_act:
                    nc.scalar.copy(
                        out=dst_fm[:, :, t * 128:(t + 1) * 128], in_=tr)
                else:
                    nc.vector.tensor_copy(
                        out=dst_fm[:, :, t * 128:(t + 1) * 128], in_=tr)

            wfc1_pool = st.enter_context(tc.tile_pool(name="wfc1", bufs=2))

            # ============ attention region (qkv + attention + proj) =========
            with AttnPools(tc) as (qk_pool, v_pool, ctx_pool, wproj_pool):
                qk_fm = qk_pool.tile([128, 2 * FC, N_TOK], BF16)
                # v: [128 key, head, key-chunk, 64+1(den)] padded to VPAD for
                # 16B-aligned DoubleRow pair strides
                v_aug = v_pool.tile([128, HEADS, TC, VPAD], F8)
                ctx_fm = ctx_pool.tile([128, FC, N_TOK], F8)
                wp = wproj_pool.tile([128, FC, DIM], F8)

                with QkvPools(tc) as (hfm_pool, wqkv_pool, wv_pool, exps_pool):
                    h_fm = hfm_pool.tile([128, FC, N_TOK], F8, tag="hfm")

                    wv = [wv_sb[:, 0], wv_sb[:, 1]]

                    wq_cache = {}

                    def emit_qk_q(m, q, fast):
                        """q/k chunk m, one token half; DR when `fast`, else
                        normal-rate fp8 (identical math, 2x PE time spent
                        inside the ACT-bound exp phase to keep the HAM
                        clock-gate open)."""
                        if m not in wq_cache:
                            wt = wqkv_pool.tile([128, FC, 128], F8, tag="wqkv")
                            nc.sync.dma_start(
                                out=wt,
                                in_=qkv_w3[:, :, m * 128:(m + 1) * 128])
                            wq_cache[m] = wt
                        wt = wq_cache[m]
                        ps = psum_small.tile([128, 512], F32, tag="sm")
                        if fast:
                            for c in range(FC // 2):
                                nc.tensor.matmul(
                                    ps, wt[:, 2 * c:2 * c + 2, :],
                                    h_fm[:, 2 * c:2 * c + 2,
                                         q * 512:(q + 1) * 512],
                                    start=(c == 0), stop=(c == FC // 2 - 1),
                                    perf_mode=DR)
                        else:
                            for k in range(FC):
                                nc.tensor.matmul(
                                    ps, wt[:, k, :],
                                    h_fm[:, k, q * 512:(q + 1) * 512],
                                    start=(k == 0), stop=(k == FC - 1))
                        nc.vector.tensor_scalar_add(
                            out=qk_fm[:, m, q * 512:(q + 1) * 512], in0=ps,
                            scalar1=qkb_pm[:, m:m + 1])
                        if q == 1:
                            wq_cache.pop(m)

                    exps_ctr = [0]

                    def alloc_exps():
                        out = []
                        for ab in range(2):
                            row = []
                            for kcp in range(TC // 2):
                                exps_ctr[0] += 1
                                row.append(exps_pool.tile(
                                    [128, 2, N_TOK], F8, tag="exps",
                                    name=f"exps{exps_ctr[0]}"))
                            out.append(row)
                        return out

                    I32 = mybir.dt.int32
                    LOG2E = 1.4426950408889634

                    def emit_exp_dve(e_out, sp):
                        """exp via float-bits construction on the DVE:
                        i = int((S*SCALE*log2e + 127 + EXP_K) * 2^23);
                        bitcast(i) ~= 2^EXP_K * exp(S*SCALE) with ~2% weight
                        error after softmax normalization (under the e4m3
                        storage error already accepted). Offloads ACT, whose
                        1 elem/lane/cycle exp stream bounds the attention
                        phase."""
                        scr = bexp_pool.tile([128, N_TOK], I32, tag="bexp")
                        nc.vector.tensor_scalar(
                            out=scr.bitcast(F32), in0=sp,
                            scalar1=float(SCALE * LOG2E * (1 << 23)),
                            scalar2=float((127.0 + EXP_K) * (1 << 23)),
                            op0=mybir.AluOpType.mult,
                            op1=mybir.AluOpType.add)
                        nc.vector.tensor_copy(out=scr, in_=scr.bitcast(F32))
                        nc.vector.tensor_copy(out=e_out, in_=scr.bitcast(F32))

                    def emit_s_exp_part(j, exps, kcp_list, q_list):
                        """S + exp for head pair (2j, 2j+1), restricted to kc
                        pairs `kcp_list` and token halves `q_list`. Half-wide
                        calls let pair 0 start as soon as LN chunks 0-3 are
                        done instead of waiting for the full LN1 sweep."""
                        for kcp in kcp_list:
                            for ab in range(2):
                                e_t = exps[ab][kcp]
                                po = 64 * ab
                                for sub in range(2):
                                    kc = 2 * kcp + sub
                                    if len(q_list) == 2:
                                        sp = psum_big.tile([128, N_TOK], F32,
                                                           tag="big")
                                        for q in q_list:
                                            nc.tensor.matmul(
                                                sp[:, q * 512:(q + 1) * 512],
                                                qk_fm[po:po + 64, 6 + j,
                                                      kc * 128:(kc + 1) * 128],
                                                qk_fm[po:po + 64, j,
                                                      q * 512:(q + 1) * 512],
                                                start=True, stop=True)
                                        if ((1 <= j <= 4 and kcp == 3
                                                and (ab == 1 or sub == 1))
                                                or (j == 5 and kcp == 0
                                                    and (ab == 1 or sub == 1))):
                                            emit_exp_dve(e_t[:, sub, :], sp)
                                        else:
                                            nc.scalar.activation(
                                                out=e_t[:, sub, :], in_=sp,
                                                func=mybir.ActivationFunctionType.Exp,
                                                scale=SCALE, bias=expk_tile)
                                    else:
                                        q = q_list[0]
                                        sp = psum_big.tile([128, 512], F32,
                                                           tag="big")
                                        nc.tensor.matmul(
                                            sp,
                                            qk_fm[po:po + 64, 6 + j,
                                                  kc * 128:(kc + 1) * 128],
                                            qk_fm[po:po + 64, j,
                                                  q * 512:(q + 1) * 512],
                                            start=True, stop=True)
                                        nc.scalar.activation(
                                            out=e_t[:, sub,
                                                    q * 512:(q + 1) * 512],
                                            in_=sp,
                                            func=mybir.ActivationFunctionType.Exp,
                                            scale=SCALE, bias=expk_tile)

                    # LN1 stats front pass: all bn_stats/aggr on DVE, then
                    # ONE batched Sqrt + ONE batched reciprocal, so every
                    # Sqrt precedes the first attention Exp (one ACT
                    # table-set switch instead of per-chunk thrash)
                    lnstat = const_pool.tile([128, TC, 3], F32)
                    for t in range(TC):
                        stats = stat_pool.tile([128, 2, 6], F32,
                                               tag="ln_stats")
                        nc.vector.bn_stats(out=stats[:, 0, :],
                                           in_=x_sb[:, t, 0:512])
                        nc.vector.bn_stats(out=stats[:, 1, :],
                                           in_=x_sb[:, t, 512:768])
                        nc.vector.bn_aggr(out=lnstat[:, t, 0:2], in_=stats)
                    nc.scalar.activation(
                        out=lnstat[:, :, 2:3], in_=lnstat[:, :, 1:2],
                        func=mybir.ActivationFunctionType.Sqrt,
                        bias=eps_tile, scale=1.0)
                    nc.vector.reciprocal(out=lnstat[:, :, 2:3],
                                         in_=lnstat[:, :, 2:3])

                    # LN1 normalize + transpose + v per token chunk
                    for t in range(TC):
                        h_tile = stat_pool.tile([128, DIM], BF16,
                                                tag="h_tile")
                        nc.vector.tensor_scalar(
                            out=h_tile, in0=x_sb[:, t, :],
                            scalar1=lnstat[:, t, 0:1],
                            scalar2=lnstat[:, t, 2:3],
                            op0=mybir.AluOpType.subtract,
                            op1=mybir.AluOpType.mult)
                        tr = psum_big.tile([128, FC, 128], BF16, tag="big")
                        for f in range(FC):
                            nc.tensor.transpose(
                                tr[:, f, :], h_tile[:, f * 128:(f + 1) * 128],
                                ident)
                        nc.scalar.copy(
                            out=h_fm[:, :, t * 128:(t + 1) * 128], in_=tr)
                        for nv in range(2):
                            ps = psum_small.tile([128, 384], F32, tag="sm")
                            for c in range(FC // 2):
                                nc.tensor.matmul(
                                    ps,
                                    h_fm[:, 2 * c:2 * c + 2,
                                         t * 128:(t + 1) * 128],
                                    wv[nv][:, 2 * c:2 * c + 2, :],
                                    start=(c == 0), stop=(c == FC // 2 - 1),
                                    perf_mode=DR)
                            nc.vector.tensor_tensor(
                                out=v_aug[:, nv * 6:(nv + 1) * 6, t, 0:HD],
                                in0=ps.rearrange("p (h d) -> p h d", d=HD),
                                in1=vb_bc[:, nv * 384:(nv + 1) * 384].rearrange(
                                    "p (h d) -> p h d", d=HD),
                                op=mybir.AluOpType.add)
                        if t == 3:
                            emit_qk_q(0, 0, True)
                            emit_qk_q(6, 0, True)
                            exps0 = alloc_exps()
                            emit_s_exp_part(0, exps0, (0, 1), (0,))
                    nc.vector.memset(v_aug[:, :, :, HD], 1.0)

                    def emit_ctx(j, exps):
                        for ab in range(2):
                            h = 2 * j + ab
                            po = 64 * ab
                            for q in range(2):
                                cp = psum_ctx.tile([65, 512], F32, tag="ctx")
                                for kc in range(TC):
                                    nc.tensor.matmul(
                                        cp,
                                        v_aug[:, h, kc, 0:65],
                                        exps[ab][kc // 2][:, kc % 2,
                                                          q * 512:(q + 1) * 512],
                                        start=(kc == 0), stop=(kc == TC - 1))
                                den = dsm_pool.tile([1, 512], F32, tag="den")
                                nc.vector.tensor_copy(out=den,
                                                      in_=cp[64:65, :])
                                scr = dsm_pool.tile([1, 512], F32, tag="scr")
                                rec = dsm_pool.tile([1, 512], F32, tag="rec")
                                nc.vector.reciprocal_approx_accurate(
                                    out=rec, in_=den, scratch=scr)
                                bcd = dsm_pool.tile([128, 512], F32, tag="bcd")
                                nc.gpsimd.partition_broadcast(bcd, rec)
                                if ab == 0:
                                    nc.vector.tensor_tensor(
                                        out=ctx_fm[0:64, j,
                                                   q * 512:(q + 1) * 512],
                                        in0=cp[0:64, :], in1=bcd[0:64, :],
                                        op=mybir.AluOpType.mult)
                                else:
                                    cu = dsm_pool.tile([128, 512], BF16,
                                                       tag="cu")
                                    nc.vector.tensor_copy(out=cu[po:po + 64, :],
                                                          in_=cp[0:64, :])
                                    nc.vector.tensor_tensor(
                                        out=ctx_fm[po:po + 64, j,
                                                   q * 512:(q + 1) * 512],
                                        in0=cu[po:po + 64, :],
                                        in1=bcd[po:po + 64, :],
                                        op=mybir.AluOpType.mult)

                    pend = None
                    for j in range(6):
                        if j == 0:
                            emit_qk_q(0, 1, True)
                            emit_qk_q(6, 1, True)
                        else:
                            emit_qk_q(j, 0, False)
                            emit_qk_q(j, 1, False)
                            emit_qk_q(6 + j, 0, False)
                            emit_qk_q(6 + j, 1, False)
                        if j == 3:
                            nc.sync.dma_start(out=wp, in_=proj_w3)
                            nc.sync.dma_start(out=projb_row,
                                              in_=row_ap(projb_d.ap(), DIM))
                        if j == 4:
                            nc.sync.dma_start(out=wf2, in_=fc2_w3)
                            nc.sync.dma_start(out=fc2b_row,
                                              in_=row_ap(fc2b64_d.ap(), DIM))
                        if j == 0:
                            emit_s_exp_part(0, exps0, (0, 1), (1,))
                            emit_s_exp_part(0, exps0, (2, 3), (0, 1))
                            exps = exps0
                        else:
                            exps = alloc_exps()
                            emit_s_exp_part(j, exps, (0, 1, 2, 3), (0, 1))
                        if pend is not None:
                            emit_ctx(*pend)
                        pend = (j, exps)
                    emit_ctx(*pend)

                # keep-warm bridge: PE idles ~3us on the last ctx evacuation
                # chain; stay busy so HAM holds 2.4 GHz into the proj sweep
                wba = psum_big.tile([128, N_TOK], F32, tag="big")
                for _ in range(8):
                    nc.tensor.matmul(wba[:, 0:512], ident, qk_fm[:, 0, 0:512],
                                     start=True, stop=True)

                # ---------------- proj + residual + LN2 ----------------
                for t in range(TC):
                    for nv in range(2):
                        ps = psum_small.tile([128, 384], F32, tag="sm")
                        # normal-rate fp8 (not DR): this region is gated by
                        # the DVE LN2/evac chain; the extra PE time keeps HAM
                        # from re-throttling into the fc1 sweep
                        for k in range(FC):
                            nc.tensor.matmul(
                                ps,
                                ctx_fm[:, k, t * 128:(t + 1) * 128],
                                wp[:, k, nv * 384:(nv + 1) * 384],
                                start=(k == 0), stop=False)
                        sl = slice(nv * 384, (nv + 1) * 384)
                        nc.tensor.matmul(
                            ps, ones_bf[0:1, :], projb_row[0:1, sl],
                            start=False, stop=True)
                        nc.vector.tensor_add(
                            out=x_sb[:, t, sl], in0=ps, in1=x_sb[:, t, sl])
                    ln_transpose(t, h2_fm, evac_act=True)

                # (warm bridge) the PE would idle >3.4us here waiting for the
                # LN2 tail, dropping HAM to 1.2 GHz for the start of fc1;
                # burn the wait on throwaway matmuls instead
                wb = psum_big.tile([128, N_TOK], F32, tag="big")
                for _ in range(16):
                    nc.tensor.matmul(wb[:, 0:512], ident, qk_fm[:, 0, 0:512],
                                     start=True, stop=True)

            # ------- MLP: fc1 (fp8 DR, x64 weights; batched fetches) -------
            if True:
                for m4 in range(MC_H // 4):
                    w1t = wfc1_pool.tile([128, FC, 512], F8, tag="w1t")
                    nc.sync.dma_start(
                        out=w1t, in_=fc1_w3[:, :, m4 * 512:(m4 + 1) * 512])
                    for mi in range(4):
                        m = 4 * m4 + mi
                        ps = psum_big.tile([128, N_TOK], F32, tag="big")
                        for half in range(2):
                            for c in range(FC // 2):
                                nc.tensor.matmul(
                                    ps[:, half * 512:(half + 1) * 512],
                                    w1t[:, 2 * c:2 * c + 2,
                                        mi * 128:(mi + 1) * 128],
                                    h2_fm[:, 2 * c:2 * c + 2,
                                          half * 512:(half + 1) * 512],
                                    start=(c == 0), stop=(c == FC // 2 - 1),
                                    perf_mode=DR)
                        nc.scalar.activation(
                            out=g_fm[:, m, :], in_=ps,
                            func=mybir.ActivationFunctionType.Gelu,
                            bias=fc1b_pm[:, m:m + 1], scale=1.0 / FC1_WS)


